# revision 26
# baseline (speedup 1.0000x reference)
"""MoE layer (top-2 of 8 experts, SwiGLU FFN) on 8 Trainium2 NeuronCores.

F-sharded ("tensor parallel over d_ff") layout: core m holds rows
[m*F/8, (m+1)*F/8) of every expert's W1/W3/W2.  Host computes the (tiny)
router matmul + top-2 dispatch once, lays the dispatched tokens out
expert-contiguously (transposed, bf16), and ships the SAME dispatch to all
cores.  Each core then runs, for every expert e and its token columns:

    h.T = W1e_slice.T @ Xe.T ; g.T = W3e_slice.T @ Xe.T   (contract over D)
    a.T = silu(h.T) * g.T                                 (ACT + DVE)
    y_part.T = W2e_slice.T @ a.T                          (contract over F/8)

so per-core work is 3*D*(F/8)*sum_e C_e MACs -- exactly balanced no matter
how skewed the routing is (unlike expert-parallel, which pays for the
largest expert on every core).  All matmuls are bf16 (1 PE cycle/row, half
the HBM traffic of fp32; rel err ~4e-3 << the 2e-2 gate).  Host sums the 8
partial outputs, applies router probabilities, and scatter-adds into the
full [B,S,D] output.

Schedule notes (from perfetto/NTFF analysis):
 - experts processed smallest-first (cheap startup fetch), second-smallest
   last (small final y store), largest in the middle
 - the first expert's weights stream as per-f panels, so the first matmul
   group is gated on ~1.2MB, not 3MB
 - throwaway warm-up matmuls on a zeroed tile keep the PE busy (and its
   p-state ramping) while the first real transfers land
 - one large contiguous DMA per (expert, tensor) otherwise: 4-8KB
   per-partition lines (DMA-issue instructions cost ~0.6us each on the
   issuing queue, so few/large transfers win); input prefetch on the SP
   queue, output stores on the Activation queue (stores waiting on
   compute must never head-of-line-block prefetch)
 - token-column padding is a multiple of 8: bf16 rows must stay
   16B-aligned in SBUF or every matmul streams ~13% slower
 - PSUM->SBUF drains alternate DVE/ACT so neither queue lags the PE and
   backs up into PSUM-buffer reuse; stage 2 borrows the stage-1 PSUM
   pools (idle during stage 2) for an 8-bank-deep accumulator rotation
 - software pipelining: stage 2 of expert e issues after stage 1 of
   expert e+1
"""

import numpy as np
import ml_dtypes

import concourse.tile as tile
from concourse import bacc, mybir
from concourse.bass_utils import run_bass_kernel_spmd

N_CORES = 8
P = 128        # SBUF partitions / matmul tile edge
BLK_MAX = 512  # max moving-dim per matmul (1 PSUM bank of fp32)
N_WARMUP = 6   # PE warm-up matmuls issued while the first DMAs land

# Results of the most recent device run (for the test harness / profiling).
last_results = None

_NC_CACHE = {}
BF16 = ml_dtypes.bfloat16


def _blocks(width):
    """Split `width` columns into near-equal blocks of <= BLK_MAX, each a
    multiple of 8 except possibly the last."""
    nb = -(-width // BLK_MAX)
    base = -(-width // (nb * 8)) * 8
    offs = []
    o = 0
    while o < width:
        w = min(base, width - o)
        offs.append((o, w))
        o += w
    return offs


def _chunks(cw, first):
    """x-transfer chunks of one expert.  Only the first processed expert
    is fetched as two half-chunks (so the first matmul group is gated on
    half the bytes); DMA-issue instructions cost ~0.6us each on the SP
    queue, so everyone else gets one large transfer."""
    if first and cw >= 256:
        h = (cw // 16) * 8
        return ((0, h), (h, cw - h))
    return ((0, cw),)


def _build_nc(ko, flt, dt, groups):
    """Device program: per-expert SwiGLU FFN over this core's F-slice.

    ko = D/128 stage-1 contraction tiles, flt = (F/8)/128 stage-1 psum
    tiles (= stage-2 contraction tiles), dt = D/128 stage-2 psum tiles.
    groups = tuple of (col_offset, padded_count, chunks) per expert in
    processing order; chunks = ((chunk_off, chunk_width), ...).  C = sum
    of padded counts.  Zero-token experts are dropped by the host.
    """
    C = sum(g[1] for g in groups)
    f32 = mybir.dt.float32
    bf16 = mybir.dt.bfloat16
    silu = mybir.ActivationFunctionType.Silu
    copy = mybir.ActivationFunctionType.Copy
    E = len(groups)

    nc = bacc.Bacc("TRN2", target_bir_lowering=False, debug=False,
                   num_devices=N_CORES)
    # expert-major packings; every per-expert transfer is one contiguous
    # [128, big] DMA
    xt_d = nc.dram_tensor("xt", [P, ko * C], bf16, kind="ExternalInput")
    w1_d = nc.dram_tensor("w1t", [E, P, flt * ko * P], bf16,
                          kind="ExternalInput")
    w3_d = nc.dram_tensor("w3t", [E, P, flt * ko * P], bf16,
                          kind="ExternalInput")
    w2_d = nc.dram_tensor("w2t", [E, P, dt * flt * P], bf16,
                          kind="ExternalInput")
    yt_d = nc.dram_tensor("yt", [P, dt * C], bf16, kind="ExternalOutput")

    cmax = max(g[1] for g in groups)

    with tile.TileContext(nc) as tc:
        with (
            tc.tile_pool(name="warmpool", bufs=1) as warmpool,
            tc.tile_pool(name="xpool", bufs=4) as xpool,
            tc.tile_pool(name="w1pool", bufs=3) as w1pool,
            tc.tile_pool(name="w3pool", bufs=3) as w3pool,
            tc.tile_pool(name="w2pool", bufs=3) as w2pool,
            tc.tile_pool(name="actpool", bufs=3) as actpool,
            tc.tile_pool(name="hpool", bufs=4) as hpool,
            tc.tile_pool(name="ypool", bufs=3) as ypool,
            tc.tile_pool(name="psh", bufs=4, space="PSUM") as psh,
            tc.tile_pool(name="psg", bufs=4, space="PSUM") as psg,
        ):
            acts = {}     # e -> act tile
            w2_sbs = {}   # e -> prefetched stage-2 weight tile
            drain_rr = [0]

            def drain(dst, src):
                # alternate PSUM->SBUF drains between DVE and ACT so
                # neither engine's queue serializes stage 2
                r = drain_rr[0] = (drain_rr[0] + 1) % 2
                if r == 0:
                    nc.vector.tensor_copy(dst, src)
                else:
                    nc.scalar.activation(dst, src, copy)

            # PE warm-up: matmuls over a zeroed tile with no transfer
            # dependencies; they execute while the first real DMAs land,
            # so the real work starts on a fully ramped PE
            warm = warmpool.tile([P, BLK_MAX], bf16)
            nc.vector.memset(warm[:], 0)
            for i in range(N_WARMUP):
                if i % 2 == 0:
                    pw = psh.tile([P, BLK_MAX], f32, tag="ph")
                else:
                    pw = psg.tile([P, BLK_MAX], f32, tag="pg")
                nc.tensor.matmul(pw[:], warm[:, :P], warm[:],
                                 start=True, stop=True)

            def stage1(e):
                off_e, cw, chunks = groups[e]
                xes = []
                for (co, cwid) in chunks:
                    xe = xpool.tile([P, ko * cwid], bf16, tag="xe")
                    nc.sync.dma_start(
                        out=xe[:],
                        in_=xt_d[:, ko * (off_e + co):
                                 ko * (off_e + co + cwid)])
                    xes.append(xe)
                w1_sb = w1pool.tile([P, flt * ko * P], bf16)
                w3_sb = w3pool.tile([P, flt * ko * P], bf16)
                # stream the first expert's weights in f-panel granules so
                # the first matmul groups aren't gated on the full expert
                nw = flt if e == 0 else 1
                for i in range(nw):
                    sl = slice(i * (flt // nw) * ko * P,
                               (i + 1) * (flt // nw) * ko * P)
                    nc.sync.dma_start(out=w1_sb[:, sl], in_=w1_d[e][:, sl])
                    nc.sync.dma_start(out=w3_sb[:, sl], in_=w3_d[e][:, sl])
                act = actpool.tile([P, flt * cmax], bf16)
                acts[e] = act
                for f in range(flt):
                    for xe, (co, cwid) in zip(xes, chunks):
                        for (bo, bw) in _blocks(cwid):
                            ph = psh.tile([P, bw], f32, tag="ph")
                            pg = psg.tile([P, bw], f32, tag="pg")
                            for k in range(ko):
                                rhs = xe[:, k * cwid + bo:k * cwid + bo + bw]
                                nc.tensor.matmul(
                                    ph[:], w1_sb[:, (f * ko + k) * P:
                                                 (f * ko + k + 1) * P],
                                    rhs, start=(k == 0), stop=(k == ko - 1))
                            for k in range(ko):
                                rhs = xe[:, k * cwid + bo:k * cwid + bo + bw]
                                nc.tensor.matmul(
                                    pg[:], w3_sb[:, (f * ko + k) * P:
                                                 (f * ko + k + 1) * P],
                                    rhs, start=(k == 0), stop=(k == ko - 1))
                            sh = hpool.tile([P, bw], bf16)
                            nc.scalar.activation(sh[:], ph[:], silu)
                            nc.vector.tensor_mul(
                                act[:, f * cmax + co + bo:
                                    f * cmax + co + bo + bw],
                                sh[:], pg[:])
                # prefetch this expert's stage-2 weights now: they're used
                # after stage 1 of the NEXT expert
                w2_sb = w2pool.tile([P, dt * flt * P], bf16)
                nc.sync.dma_start(out=w2_sb[:], in_=w2_d[e])
                w2_sbs[e] = w2_sb

            def stage2(e):
                off_e, cw, chunks = groups[e]
                act = acts.pop(e)
                w2_sb = w2_sbs.pop(e)
                # the last expert streams its output per d-tile so the
                # final store is tiny
                step = 1 if e == E - 1 else dt // 2
                for (co, cwid) in chunks:
                    for (bo0, bw) in _blocks(cwid):
                        bo = co + bo0
                        y_sb = ypool.tile([P, dt * bw], bf16, tag="y_sb")
                        for d in range(dt):
                            # borrow the (idle) stage-1 PSUM pools: 8-deep
                            # accumulator rotation
                            if d % 2 == 0:
                                py = psh.tile([P, bw], f32, tag="ph")
                            else:
                                py = psg.tile([P, bw], f32, tag="pg")
                            for f in range(flt):
                                nc.tensor.matmul(
                                    py[:], w2_sb[:, (d * flt + f) * P:
                                                 (d * flt + f + 1) * P],
                                    act[:, f * cmax + bo:f * cmax + bo + bw],
                                    start=(f == 0), stop=(f == flt - 1))
                            drain(y_sb[:, d * bw:(d + 1) * bw], py[:])
                            if d % step == step - 1:
                                # store finished d-range while the rest
                                # computes.  Normally on the Activation
                                # queue (keeps the SP prefetch queue
                                # clean); the last expert uses the SP
                                # queue, which is idle by then, so final
                                # stores don't serialize behind the last
                                # ACT drains
                                eng = nc.sync if e == E - 1 else nc.scalar
                                lo = (d + 1 - step) * bw
                                hi = (d + 1) * bw
                                eng.dma_start(
                                    out=yt_d[:, dt * (off_e + bo) + lo:
                                             dt * (off_e + bo) + hi],
                                    in_=y_sb[:, lo:hi])

            # software-pipelined order: stage 2 of expert e issues after
            # stage 1 of expert e+1, so the PE never waits on the drain
            # tail of the expert it just finished
            stage1(0)
            for e in range(1, E):
                stage1(e)
                stage2(e - 1)
            stage2(E - 1)
    nc.compile()
    return nc


def _route(xt, Wr):
    """Replicate the reference's top-2 routing on host (fp32).

    Selection is robust: 2nd/3rd logit gaps are >> fp32 matmul noise.
    Stable argsort on -logits matches jax.lax.top_k tie-breaking
    (lower index first on exact ties).
    """
    logits = xt @ Wr                                     # [T, E] f32
    order = np.argsort(-logits, axis=1, kind="stable")[:, :2]
    v = np.take_along_axis(logits, order, axis=1)
    ex = np.exp(v - v[:, :1])
    probs = ex / ex.sum(axis=1, keepdims=True)           # [T, 2] f32
    return order, probs


def kernel(x, Wr, W1, W2, W3):
    global last_results
    x = np.asarray(x)
    Wr, W1, W2, W3 = (np.asarray(a) for a in (Wr, W1, W2, W3))
    b, s, D = x.shape
    E = Wr.shape[1]
    F = W1.shape[2]
    T = b * s
    FL = F // N_CORES                      # per-core F slice
    ko, flt, dt = D // P, FL // P, D // P

    xt = np.ascontiguousarray(x.reshape(T, D), dtype=np.float32)
    order, probs = _route(xt, Wr)

    idx = [np.nonzero((order == e).any(axis=1))[0] for e in range(E)]
    live = [e for e in range(E) if len(idx[e])]
    # processing order: smallest expert first (cheap startup fetch),
    # second-smallest last (small final store), rest descending between
    by_size = sorted(live, key=lambda e: len(idx[e]))
    if len(by_size) > 2:
        live = [by_size[0]] + by_size[:1:-1] + [by_size[1]]
    else:
        live = by_size
    nl = len(live)
    # pad to 8 columns: bf16 rows must stay 16B-aligned in SBUF, and
    # matmul moving-dim widths that break 16B alignment stream ~13% slower
    pads = [-(-len(idx[e]) // 8) * 8 for e in live]
    offs = np.concatenate([[0], np.cumsum(pads)])
    groups = tuple(
        (int(offs[i]), int(pads[i]), _chunks(int(pads[i]), i == 0))
        for i in range(nl))
    C = int(offs[-1])

    key = (ko, flt, dt, groups)
    if key not in _NC_CACHE:
        _NC_CACHE[key] = _build_nc(ko, flt, dt, groups)
    nc = _NC_CACHE[key]

    # dispatched tokens: expert-major, transposed, k-major within chunk
    xd = np.zeros((P, ko * C), dtype=BF16)
    for i, e in enumerate(live):
        ids = idx[e]
        off_e, cw, chunks = groups[i]
        xe = np.zeros((D, cw), dtype=BF16)
        xe[:, :len(ids)] = xt[ids].T.astype(BF16)
        for (co, cwid) in chunks:
            xd[:, ko * (off_e + co):ko * (off_e + co + cwid)] = (
                xe[:, co:co + cwid].reshape(ko, P, cwid)
                .transpose(1, 0, 2).reshape(P, ko * cwid))

    W1b, W3b, W2b = (w.astype(BF16) for w in (W1, W3, W2))
    in_maps = []
    for m in range(N_CORES):
        sl = slice(m * FL, (m + 1) * FL)
        w1t = np.ascontiguousarray(
            W1b[live, :, sl].reshape(nl, ko, P, flt, P)
            .transpose(0, 2, 3, 1, 4)).reshape(nl, P, flt * ko * P)
        w3t = np.ascontiguousarray(
            W3b[live, :, sl].reshape(nl, ko, P, flt, P)
            .transpose(0, 2, 3, 1, 4)).reshape(nl, P, flt * ko * P)
        w2t = np.ascontiguousarray(
            W2b[live, sl, :].reshape(nl, flt, P, dt, P)
            .transpose(0, 2, 3, 1, 4)).reshape(nl, P, dt * flt * P)
        in_maps.append({"xt": xd, "w1t": w1t, "w3t": w3t, "w2t": w2t})

    res = run_bass_kernel_spmd(nc, in_maps, core_ids=list(range(N_CORES)))
    last_results = res

    # sum the per-core partial outputs (each covers F/8 of the contraction)
    yt = sum(res.results[m]["yt"].astype(np.float64) for m in range(N_CORES))
    y = np.empty((D, C), dtype=np.float64)
    for i in range(nl):
        off_e, cw, chunks = groups[i]
        for (co, cwid) in chunks:
            for (bo0, bw) in _blocks(cwid):
                bo = co + bo0
                seg = yt[:, dt * (off_e + bo):dt * (off_e + bo) + dt * bw]
                y[:, off_e + bo:off_e + bo + bw] = (
                    seg.reshape(P, dt, bw).transpose(1, 0, 2).reshape(D, bw))

    out = np.zeros((T, D), dtype=np.float64)
    for i, e in enumerate(live):
        ids = idx[e]
        ye = y[:, offs[i]:offs[i] + len(ids)]            # [D, Ne]
        slot = (order[ids] == e).argmax(axis=1)
        pe = probs[ids, slot].astype(np.float64)
        out[ids] += ye.T * pe[:, None]
    return out.astype(np.float32).reshape(b, s, D)


# revision 29
# speedup vs baseline: 1.0141x; 1.0141x over previous
"""MoE layer (top-2 of 8 experts, SwiGLU FFN) on 8 Trainium2 NeuronCores.

F-sharded ("tensor parallel over d_ff") layout: core m holds rows
[m*F/8, (m+1)*F/8) of every expert's W1/W3/W2.  Host computes the (tiny)
router matmul + top-2 dispatch once, lays the dispatched tokens out
expert-contiguously (transposed, bf16), and ships the SAME dispatch to all
cores.  Each core then runs, for every expert e and its token columns:

    h.T = W1e_slice.T @ Xe.T ; g.T = W3e_slice.T @ Xe.T   (contract over D)
    a.T = silu(h.T) * g.T                                 (ACT + DVE)
    y_part.T = W2e_slice.T @ a.T                          (contract over F/8)

so per-core work is 3*D*(F/8)*sum_e C_e MACs -- exactly balanced no matter
how skewed the routing is (unlike expert-parallel, which pays for the
largest expert on every core).  All matmuls are bf16 (1 PE cycle/row, half
the HBM traffic of fp32; rel err ~4e-3 << the 2e-2 gate).  Host sums the 8
partial outputs, applies router probabilities, and scatter-adds into the
full [B,S,D] output.

Schedule notes (from perfetto/NTFF analysis):
 - experts processed smallest-first (cheap startup fetch), second-smallest
   last (small final y store), largest in the middle
 - the first expert's weights stream as per-f panels, so the first matmul
   group is gated on ~1.2MB, not 3MB
 - throwaway warm-up matmuls on a zeroed tile keep the PE busy (and its
   p-state ramping) while the first real transfers land
 - one large contiguous DMA per (expert, tensor) otherwise: 4-8KB
   per-partition lines (DMA-issue instructions cost ~0.6us each on the
   issuing queue, so few/large transfers win); input prefetch on the SP
   queue, output stores on the Activation queue (stores waiting on
   compute must never head-of-line-block prefetch)
 - token-column padding is a multiple of 8: bf16 rows must stay
   16B-aligned in SBUF or every matmul streams ~13% slower
 - PSUM->SBUF drains alternate DVE/ACT so neither queue lags the PE and
   backs up into PSUM-buffer reuse; stage 2 borrows the stage-1 PSUM
   pools (idle during stage 2) for an 8-bank-deep accumulator rotation
 - software pipelining: stage 2 of expert e issues after stage 1 of
   expert e+1
"""

import numpy as np
import ml_dtypes

import concourse.tile as tile
from concourse import bacc, mybir
from concourse.bass_utils import run_bass_kernel_spmd

N_CORES = 8
P = 128        # SBUF partitions / matmul tile edge
BLK_MAX = 512  # max moving-dim per matmul (1 PSUM bank of fp32)
N_WARMUP = 6   # PE warm-up matmuls issued while the first DMAs land

# Results of the most recent device run (for the test harness / profiling).
last_results = None

_NC_CACHE = {}
BF16 = ml_dtypes.bfloat16


def _blocks(width):
    """Split `width` columns into near-equal blocks of <= BLK_MAX, each a
    multiple of 8 except possibly the last."""
    nb = -(-width // BLK_MAX)
    base = -(-width // (nb * 8)) * 8
    offs = []
    o = 0
    while o < width:
        w = min(base, width - o)
        offs.append((o, w))
        o += w
    return offs


def _chunks(cw, first):
    """x-transfer chunks of one expert (single chunk: DMA-issue
    instructions are expensive on the SP queue, so fewer, larger
    transfers win; a split first expert measured no faster)."""
    return ((0, cw),)


def _build_nc(ko, flt, dt, groups):
    """Device program: per-expert SwiGLU FFN over this core's F-slice.

    ko = D/128 stage-1 contraction tiles, flt = (F/8)/128 stage-1 psum
    tiles (= stage-2 contraction tiles), dt = D/128 stage-2 psum tiles.
    groups = tuple of (col_offset, padded_count, chunks) per expert in
    processing order; chunks = ((chunk_off, chunk_width), ...).  C = sum
    of padded counts.  Zero-token experts are dropped by the host.
    """
    C = sum(g[1] for g in groups)
    f32 = mybir.dt.float32
    bf16 = mybir.dt.bfloat16
    silu = mybir.ActivationFunctionType.Silu
    copy = mybir.ActivationFunctionType.Copy
    E = len(groups)

    nc = bacc.Bacc("TRN2", target_bir_lowering=False, debug=False,
                   num_devices=N_CORES)
    # expert-major packings; every per-expert transfer is one contiguous
    # [128, big] DMA
    xt_d = nc.dram_tensor("xt", [P, ko * C], bf16, kind="ExternalInput")
    w1_d = nc.dram_tensor("w1t", [E, P, flt * ko * P], bf16,
                          kind="ExternalInput")
    w3_d = nc.dram_tensor("w3t", [E, P, flt * ko * P], bf16,
                          kind="ExternalInput")
    w2_d = nc.dram_tensor("w2t", [E, P, dt * flt * P], bf16,
                          kind="ExternalInput")
    yt_d = nc.dram_tensor("yt", [P, dt * C], bf16, kind="ExternalOutput")

    cmax = max(g[1] for g in groups)

    with tile.TileContext(nc) as tc:
        with (
            tc.tile_pool(name="warmpool", bufs=1) as warmpool,
            tc.tile_pool(name="xpool", bufs=4) as xpool,
            tc.tile_pool(name="w1pool", bufs=3) as w1pool,
            tc.tile_pool(name="w3pool", bufs=3) as w3pool,
            tc.tile_pool(name="w2pool", bufs=3) as w2pool,
            tc.tile_pool(name="actpool", bufs=3) as actpool,
            tc.tile_pool(name="hpool", bufs=4) as hpool,
            tc.tile_pool(name="ypool", bufs=3) as ypool,
            tc.tile_pool(name="psh", bufs=4, space="PSUM") as psh,
            tc.tile_pool(name="psg", bufs=4, space="PSUM") as psg,
        ):
            acts = {}     # e -> act tile
            w2_sbs = {}   # e -> prefetched stage-2 weight tile
            drain_rr = [0]

            def drain(dst, src):
                # alternate PSUM->SBUF drains between DVE and ACT so
                # neither engine's queue serializes stage 2
                r = drain_rr[0] = (drain_rr[0] + 1) % 2
                if r == 0:
                    nc.vector.tensor_copy(dst, src)
                else:
                    nc.scalar.activation(dst, src, copy)

            # PE warm-up: matmuls over a zeroed tile with no transfer
            # dependencies; they execute while the first real DMAs land,
            # so the real work starts on a fully ramped PE
            warm = warmpool.tile([P, BLK_MAX], bf16)
            nc.vector.memset(warm[:], 0)
            for i in range(N_WARMUP):
                if i % 2 == 0:
                    pw = psh.tile([P, BLK_MAX], f32, tag="ph")
                else:
                    pw = psg.tile([P, BLK_MAX], f32, tag="pg")
                nc.tensor.matmul(pw[:], warm[:, :P], warm[:],
                                 start=True, stop=True)

            def stage1(e):
                off_e, cw, chunks = groups[e]
                xes = []
                for (co, cwid) in chunks:
                    xe = xpool.tile([P, ko * cwid], bf16, tag="xe")
                    nc.sync.dma_start(
                        out=xe[:],
                        in_=xt_d[:, ko * (off_e + co):
                                 ko * (off_e + co + cwid)])
                    xes.append(xe)
                w1_sb = w1pool.tile([P, flt * ko * P], bf16)
                w3_sb = w3pool.tile([P, flt * ko * P], bf16)
                # stream the first expert's weights in f-panel granules so
                # the first matmul groups aren't gated on the full expert
                nw = flt if e == 0 else 1
                for i in range(nw):
                    sl = slice(i * (flt // nw) * ko * P,
                               (i + 1) * (flt // nw) * ko * P)
                    nc.sync.dma_start(out=w1_sb[:, sl], in_=w1_d[e][:, sl])
                    nc.sync.dma_start(out=w3_sb[:, sl], in_=w3_d[e][:, sl])
                act = actpool.tile([P, flt * cmax], bf16)
                acts[e] = act
                for f in range(flt):
                    for xe, (co, cwid) in zip(xes, chunks):
                        for (bo, bw) in _blocks(cwid):
                            ph = psh.tile([P, bw], f32, tag="ph")
                            pg = psg.tile([P, bw], f32, tag="pg")
                            for k in range(ko):
                                rhs = xe[:, k * cwid + bo:k * cwid + bo + bw]
                                nc.tensor.matmul(
                                    ph[:], w1_sb[:, (f * ko + k) * P:
                                                 (f * ko + k + 1) * P],
                                    rhs, start=(k == 0), stop=(k == ko - 1))
                            for k in range(ko):
                                rhs = xe[:, k * cwid + bo:k * cwid + bo + bw]
                                nc.tensor.matmul(
                                    pg[:], w3_sb[:, (f * ko + k) * P:
                                                 (f * ko + k + 1) * P],
                                    rhs, start=(k == 0), stop=(k == ko - 1))
                            sh = hpool.tile([P, bw], bf16)
                            nc.scalar.activation(sh[:], ph[:], silu)
                            nc.vector.tensor_mul(
                                act[:, f * cmax + co + bo:
                                    f * cmax + co + bo + bw],
                                sh[:], pg[:])
                # prefetch this expert's stage-2 weights now: they're used
                # after stage 1 of the NEXT expert
                w2_sb = w2pool.tile([P, dt * flt * P], bf16)
                nc.sync.dma_start(out=w2_sb[:], in_=w2_d[e])
                w2_sbs[e] = w2_sb

            def stage2(e):
                off_e, cw, chunks = groups[e]
                act = acts.pop(e)
                w2_sb = w2_sbs.pop(e)
                # the last expert streams its output in d-pair granules so
                # the final store is tiny
                step = 2 if e == E - 1 else dt // 2
                for (co, cwid) in chunks:
                    for (bo0, bw) in _blocks(cwid):
                        bo = co + bo0
                        y_sb = ypool.tile([P, dt * bw], bf16, tag="y_sb")
                        for d in range(dt):
                            # borrow the (idle) stage-1 PSUM pools: 8-deep
                            # accumulator rotation
                            if d % 2 == 0:
                                py = psh.tile([P, bw], f32, tag="ph")
                            else:
                                py = psg.tile([P, bw], f32, tag="pg")
                            for f in range(flt):
                                nc.tensor.matmul(
                                    py[:], w2_sb[:, (d * flt + f) * P:
                                                 (d * flt + f + 1) * P],
                                    act[:, f * cmax + bo:f * cmax + bo + bw],
                                    start=(f == 0), stop=(f == flt - 1))
                            drain(y_sb[:, d * bw:(d + 1) * bw], py[:])
                            if d % step == step - 1:
                                # store finished d-range while the rest
                                # computes; Activation queue keeps the SP
                                # prefetch queue clean
                                lo = (d + 1 - step) * bw
                                hi = (d + 1) * bw
                                nc.scalar.dma_start(
                                    out=yt_d[:, dt * (off_e + bo) + lo:
                                             dt * (off_e + bo) + hi],
                                    in_=y_sb[:, lo:hi])

            # software-pipelined order: stage 2 of expert e issues after
            # stage 1 of expert e+1, so the PE never waits on the drain
            # tail of the expert it just finished
            stage1(0)
            for e in range(1, E):
                stage1(e)
                stage2(e - 1)
            stage2(E - 1)
    nc.compile()
    return nc


def _route(xt, Wr):
    """Replicate the reference's top-2 routing on host (fp32).

    Selection is robust: 2nd/3rd logit gaps are >> fp32 matmul noise.
    Stable argsort on -logits matches jax.lax.top_k tie-breaking
    (lower index first on exact ties).
    """
    logits = xt @ Wr                                     # [T, E] f32
    order = np.argsort(-logits, axis=1, kind="stable")[:, :2]
    v = np.take_along_axis(logits, order, axis=1)
    ex = np.exp(v - v[:, :1])
    probs = ex / ex.sum(axis=1, keepdims=True)           # [T, 2] f32
    return order, probs


def kernel(x, Wr, W1, W2, W3):
    global last_results
    x = np.asarray(x)
    Wr, W1, W2, W3 = (np.asarray(a) for a in (Wr, W1, W2, W3))
    b, s, D = x.shape
    E = Wr.shape[1]
    F = W1.shape[2]
    T = b * s
    FL = F // N_CORES                      # per-core F slice
    ko, flt, dt = D // P, FL // P, D // P

    xt = np.ascontiguousarray(x.reshape(T, D), dtype=np.float32)
    order, probs = _route(xt, Wr)

    idx = [np.nonzero((order == e).any(axis=1))[0] for e in range(E)]
    live = [e for e in range(E) if len(idx[e])]
    # processing order: smallest expert first (cheap startup fetch),
    # second-smallest last (small final store), rest descending between
    by_size = sorted(live, key=lambda e: len(idx[e]))
    if len(by_size) > 2:
        live = [by_size[0]] + by_size[:1:-1] + [by_size[1]]
    else:
        live = by_size
    nl = len(live)
    # pad to 8 columns: bf16 rows must stay 16B-aligned in SBUF, and
    # matmul moving-dim widths that break 16B alignment stream ~13% slower
    pads = [-(-len(idx[e]) // 8) * 8 for e in live]
    offs = np.concatenate([[0], np.cumsum(pads)])
    groups = tuple(
        (int(offs[i]), int(pads[i]), _chunks(int(pads[i]), i == 0))
        for i in range(nl))
    C = int(offs[-1])

    key = (ko, flt, dt, groups)
    if key not in _NC_CACHE:
        _NC_CACHE[key] = _build_nc(ko, flt, dt, groups)
    nc = _NC_CACHE[key]

    # dispatched tokens: expert-major, transposed, k-major within chunk
    xd = np.zeros((P, ko * C), dtype=BF16)
    for i, e in enumerate(live):
        ids = idx[e]
        off_e, cw, chunks = groups[i]
        xe = np.zeros((D, cw), dtype=BF16)
        xe[:, :len(ids)] = xt[ids].T.astype(BF16)
        for (co, cwid) in chunks:
            xd[:, ko * (off_e + co):ko * (off_e + co + cwid)] = (
                xe[:, co:co + cwid].reshape(ko, P, cwid)
                .transpose(1, 0, 2).reshape(P, ko * cwid))

    W1b, W3b, W2b = (w.astype(BF16) for w in (W1, W3, W2))
    in_maps = []
    for m in range(N_CORES):
        sl = slice(m * FL, (m + 1) * FL)
        w1t = np.ascontiguousarray(
            W1b[live, :, sl].reshape(nl, ko, P, flt, P)
            .transpose(0, 2, 3, 1, 4)).reshape(nl, P, flt * ko * P)
        w3t = np.ascontiguousarray(
            W3b[live, :, sl].reshape(nl, ko, P, flt, P)
            .transpose(0, 2, 3, 1, 4)).reshape(nl, P, flt * ko * P)
        w2t = np.ascontiguousarray(
            W2b[live, sl, :].reshape(nl, flt, P, dt, P)
            .transpose(0, 2, 3, 1, 4)).reshape(nl, P, dt * flt * P)
        in_maps.append({"xt": xd, "w1t": w1t, "w3t": w3t, "w2t": w2t})

    res = run_bass_kernel_spmd(nc, in_maps, core_ids=list(range(N_CORES)))
    last_results = res

    # sum the per-core partial outputs (each covers F/8 of the contraction)
    yt = sum(res.results[m]["yt"].astype(np.float64) for m in range(N_CORES))
    y = np.empty((D, C), dtype=np.float64)
    for i in range(nl):
        off_e, cw, chunks = groups[i]
        for (co, cwid) in chunks:
            for (bo0, bw) in _blocks(cwid):
                bo = co + bo0
                seg = yt[:, dt * (off_e + bo):dt * (off_e + bo) + dt * bw]
                y[:, off_e + bo:off_e + bo + bw] = (
                    seg.reshape(P, dt, bw).transpose(1, 0, 2).reshape(D, bw))

    out = np.zeros((T, D), dtype=np.float64)
    for i, e in enumerate(live):
        ids = idx[e]
        ye = y[:, offs[i]:offs[i] + len(ids)]            # [D, Ne]
        slot = (order[ids] == e).argmax(axis=1)
        pe = probs[ids, slot].astype(np.float64)
        out[ids] += ye.T * pe[:, None]
    return out.astype(np.float32).reshape(b, s, D)
